# revision 15
# baseline (speedup 1.0000x reference)
"""Trainium2 Bass kernel for the conv-BN-relu x3 + conv1x1 + sigmoid-weighted
3x3 WLS jet fit module.

Contract: kernel(**inputs) takes the FULL unsharded inputs (numpy arrays,
keys as in reference.setup_inputs()) and returns the FULL output tuple
(normal (512,3), beta (512,3), weights (512,256)), all float32.

Internally: data-parallel over the batch axis B=512 across 8 NeuronCores
(64 neighborhoods each). Training-mode BatchNorm statistics are combined
across cores with a per-layer AllReduce of per-channel [mean, E[z^2]].
"""
import sys
import os

sys.path.insert(0, '/opt/trn_rl_repo')

import numpy as np

import concourse.bass as bass
import concourse.mybir as mybir
import concourse.tile as tile
from concourse import bacc
from concourse.bass_utils import run_bass_kernel_spmd

F32 = mybir.dt.float32
F32R = mybir.dt.float32r
FP16 = mybir.dt.float16
AF = mybir.ActivationFunctionType
ALU = mybir.AluOpType
AX = mybir.AxisListType

NCORES = 8
B, C, N = 512, 1024, 256
BP = B // NCORES            # 64 neighborhoods per core
BN = BP * N                 # 16384 samples per core
TW = 512                    # free-dim tile width
NT = BN // TW               # 32 n-tiles per core
BPT = TW // N               # 2 neighborhoods per n-tile
EPS_BN = 1e-5
M_TOTAL = float(B * N)      # global BatchNorm sample count

# layer channel configs: (Cin, Cout)
L1 = (1024, 512)
L2 = (512, 256)
L3 = (256, 128)


def _build(stages=8):
    nc = bacc.Bacc("TRN2", target_bir_lowering=False, debug=False,
                   enable_asserts=True, num_devices=NCORES)

    # ---- kernel I/O ----
    x_in = nc.dram_tensor("x", (BP, C, N), F32R, kind="ExternalInput").ap()
    pts_in = nc.dram_tensor("points", (BP, 3, N), F32, kind="ExternalInput").ap()
    w1t_in = nc.dram_tensor("w1t", (1024, 512), F32R, kind="ExternalInput").ap()
    w2t_in = nc.dram_tensor("w2t", (512, 256), FP16, kind="ExternalInput").ap()
    w3t_in = nc.dram_tensor("w3t", (256, 128), FP16, kind="ExternalInput").ap()
    w4c_in = nc.dram_tensor("w4c", (128, 1), FP16, kind="ExternalInput").ap()
    g1_in = nc.dram_tensor("g1c", (128, 4), F32, kind="ExternalInput").ap()
    be1_in = nc.dram_tensor("be1c", (128, 4), F32, kind="ExternalInput").ap()
    g2_in = nc.dram_tensor("g2c", (128, 2), F32, kind="ExternalInput").ap()
    be2_in = nc.dram_tensor("be2c", (128, 2), F32, kind="ExternalInput").ap()
    g3_in = nc.dram_tensor("g3c", (128, 1), F32, kind="ExternalInput").ap()
    be3_in = nc.dram_tensor("be3c", (128, 1), F32, kind="ExternalInput").ap()
    b4_in = nc.dram_tensor("b4c", (BP, 1), F32, kind="ExternalInput").ap()

    nrm_out = nc.dram_tensor("normal", (BP, 3), F32, kind="ExternalOutput").ap()
    beta_out = nc.dram_tensor("beta", (BP, 3), F32, kind="ExternalOutput").ap()
    wgt_out = nc.dram_tensor("weights", (BP, N), F32, kind="ExternalOutput").ap()

    with tile.TileContext(nc) as tc:
        _emit(nc, tc, x_in, pts_in,
              w1t_in, w2t_in, w3t_in, w4c_in,
              g1_in, be1_in, g2_in, be2_in, g3_in, be3_in, b4_in,
              nrm_out, beta_out, wgt_out, stages)
    nc.compile()
    return nc


def _emit(nc, tc, x_in, pts_in, w1t_in, w2t_in, w3t_in, w4c_in,
          g1_in, be1_in, g2_in, be2_in, g3_in, be3_in, b4_in,
          nrm_out, beta_out, wgt_out, stages=4):
    from contextlib import ExitStack
    ctx = ExitStack()
    with ctx:
        wpool = ctx.enter_context(tc.tile_pool(name="wpool", bufs=1))
        xpool = ctx.enter_context(tc.tile_pool(name="xpool", bufs=12))
        zevict = ctx.enter_context(tc.tile_pool(name="zevict", bufs=6))
        zload = ctx.enter_context(tc.tile_pool(name="zload", bufs=8))
        hpool = ctx.enter_context(tc.tile_pool(name="hpool", bufs=8))
        spool = ctx.enter_context(tc.tile_pool(name="spool", bufs=1))
        tiny = ctx.enter_context(tc.tile_pool(name="tiny", bufs=1))
        scr = ctx.enter_context(tc.tile_pool(name="scr", bufs=1))
        psum = ctx.enter_context(tc.tile_pool(name="psum", bufs=8, space="PSUM"))
        dram = ctx.enter_context(tc.tile_pool(name="dram", bufs=1, space="DRAM"))

        # ---------------- weights / params load ----------------
        w1 = []
        for k in range(8):
            t = wpool.tile([128, 512], F32R, tag=f"w1_{k}")
            nc.sync.dma_start(t[:], w1t_in[k * 128:(k + 1) * 128, :])
            w1.append(t)
        w2 = []
        for k in range(4):
            t = wpool.tile([128, 256], FP16, tag=f"w2_{k}")
            nc.sync.dma_start(t[:], w2t_in[k * 128:(k + 1) * 128, :])
            w2.append(t)
        w3 = []
        for k in range(2):
            t = wpool.tile([128, 128], FP16, tag=f"w3_{k}")
            nc.sync.dma_start(t[:], w3t_in[k * 128:(k + 1) * 128, :])
            w3.append(t)
        w4 = wpool.tile([128, 1], FP16, tag="w4")
        nc.sync.dma_start(w4[:], w4c_in[:])

        g1 = wpool.tile([128, 4], F32, tag="g1")
        nc.sync.dma_start(g1[:], g1_in[:])
        be1 = wpool.tile([128, 4], F32, tag="be1")
        nc.sync.dma_start(be1[:], be1_in[:])
        g2 = wpool.tile([128, 2], F32, tag="g2")
        nc.sync.dma_start(g2[:], g2_in[:])
        be2 = wpool.tile([128, 2], F32, tag="be2")
        nc.sync.dma_start(be2[:], be2_in[:])
        g3 = wpool.tile([128, 1], F32, tag="g3")
        nc.sync.dma_start(g3[:], g3_in[:])
        be3 = wpool.tile([128, 1], F32, tag="be3")
        nc.sync.dma_start(be3[:], be3_in[:])
        b4 = wpool.tile([BP, 1], F32, tag="b4")
        nc.sync.dma_start(b4[:], b4_in[:])

        eps_t = tiny.tile([128, 1], F32, tag="eps")
        nc.vector.memset(eps_t[:], EPS_BN)

        # ---------------- DRAM scratch for z (fp16) ----------------
        z1d = [dram.tile([128, BN], FP16, name=f"z1d_{m}") for m in range(4)]
        z2d = [dram.tile([128, BN], FP16, name=f"z2d_{m}") for m in range(2)]
        z3d = [dram.tile([128, BN], FP16, name=f"z3d_{m}") for m in range(1)]
        lgd = dram.tile([1, BN], F32)

        # per-layer bn_stats collection tiles: [128, NT*6] per m-chunk
        st1 = [spool.tile([128, NT * 6], F32, name=f"st1_{m}") for m in range(4)]
        st2 = [spool.tile([128, NT * 6], F32, name=f"st2_{m}") for m in range(2)]
        st3 = [spool.tile([128, NT * 6], F32, name=f"st3_{m}") for m in range(1)]

        # =====================================================
        # Phase A1: z1 = W1 @ x ; bn stats ; store z1 (fp16)
        # =====================================================
        with nc.named_scope("phaseA1"):
            for n in range(NT):
                b0 = n * BPT
                xts = []
                for k in range(8):
                    xt = xpool.tile([128, BPT, N], F32R, tag="xt")
                    src = x_in[b0:b0 + BPT, k * 128:(k + 1) * 128, :]
                    nc.sync.dma_start(xt[:], src.rearrange("b c n -> c b n"))
                    xts.append(xt)
                for m in range(4):
                    pz = psum.tile([128, TW], F32, tag="ps")
                    for k in range(8):
                        nc.tensor.matmul(
                            pz[:],
                            lhsT=w1[k][:, m * 128:(m + 1) * 128],
                            rhs=xts[k][:].rearrange("c b n -> c (b n)"),
                            start=(k == 0), stop=(k == 7))
                    zsb = zevict.tile([128, TW], FP16, tag="zsb")
                    nc.scalar.copy(zsb[:], pz[:])
                    nc.sync.dma_start(z1d[m][:, n * TW:(n + 1) * TW], zsb[:])
                    nc.vector.bn_stats(st1[m][:, n * 6:(n + 1) * 6], pz[:])

        if stages < 2:
            _dummy_outputs(nc, tiny, nrm_out, beta_out, wgt_out)
            return
        # stats1 -> AllReduce -> scale/bias for layer1 BN
        sc1, bi1 = _bn_params(nc, tc, tiny, scr, dram, st1, g1, be1, eps_t, "bn1")

        # =====================================================
        # Phase B1: h1 = relu(bn(z1)) ; z2 = W2 @ h1 ; stats ; store z2
        # =====================================================
        with nc.named_scope("phaseB1"):
            for n in range(NT):
                hts = []
                for k in range(4):
                    zt = zload.tile([128, TW], FP16, tag="z1ld")
                    nc.sync.dma_start(zt[:], z1d[k][:, n * TW:(n + 1) * TW])
                    ht = hpool.tile([128, TW], FP16, tag="h1")
                    nc.scalar.activation(ht[:], zt[:], AF.Relu,
                                         bias=bi1[:, k:k + 1], scale=sc1[:, k:k + 1])
                    hts.append(ht)
                for m in range(2):
                    pz = psum.tile([128, TW], F32, tag="ps")
                    for k in range(4):
                        nc.tensor.matmul(
                            pz[:],
                            lhsT=w2[k][:, m * 128:(m + 1) * 128],
                            rhs=hts[k][:],
                            start=(k == 0), stop=(k == 3))
                    zsb = zevict.tile([128, TW], FP16, tag="zsb")
                    nc.vector.tensor_copy(zsb[:], pz[:])
                    nc.sync.dma_start(z2d[m][:, n * TW:(n + 1) * TW], zsb[:])
                    nc.vector.bn_stats(st2[m][:, n * 6:(n + 1) * 6], pz[:])

        if stages < 3:
            _dummy_outputs(nc, tiny, nrm_out, beta_out, wgt_out)
            return
        sc2, bi2 = _bn_params(nc, tc, tiny, scr, dram, st2, g2, be2, eps_t, "bn2")

        # =====================================================
        # Phase B2: h2 = relu(bn(z2)) ; z3 = W3 @ h2 ; stats ; store z3
        # =====================================================
        with nc.named_scope("phaseB2"):
            for n in range(NT):
                hts = []
                for k in range(2):
                    zt = zload.tile([128, TW], FP16, tag="z2ld")
                    nc.sync.dma_start(zt[:], z2d[k][:, n * TW:(n + 1) * TW])
                    ht = hpool.tile([128, TW], FP16, tag="h2")
                    nc.scalar.activation(ht[:], zt[:], AF.Relu,
                                         bias=bi2[:, k:k + 1], scale=sc2[:, k:k + 1])
                    hts.append(ht)
                pz = psum.tile([128, TW], F32, tag="ps")
                for k in range(2):
                    nc.tensor.matmul(
                        pz[:], lhsT=w3[k][:], rhs=hts[k][:],
                        start=(k == 0), stop=(k == 1))
                zsb = zevict.tile([128, TW], FP16, tag="zsb")
                nc.vector.tensor_copy(zsb[:], pz[:])
                nc.sync.dma_start(z3d[0][:, n * TW:(n + 1) * TW], zsb[:])
                nc.vector.bn_stats(st3[0][:, n * 6:(n + 1) * 6], pz[:])

        if stages < 4:
            _dummy_outputs(nc, tiny, nrm_out, beta_out, wgt_out)
            return
        sc3, bi3 = _bn_params(nc, tc, tiny, scr, dram, st3, g3, be3, eps_t, "bn3")

        # =====================================================
        # Phase B3: h3 = relu(bn(z3)) ; logits = w4 @ h3 -> DRAM row
        # =====================================================
        with nc.named_scope("phaseB3"):
            for n in range(NT):
                zt = zload.tile([128, TW], FP16, tag="z3ld")
                nc.sync.dma_start(zt[:], z3d[0][:, n * TW:(n + 1) * TW])
                ht = hpool.tile([128, TW], FP16, tag="h3")
                nc.scalar.activation(ht[:], zt[:], AF.Relu,
                                     bias=bi3[:, 0:1], scale=sc3[:, 0:1])
                pl = psum.tile([1, TW], F32, tag="ps")
                nc.tensor.matmul(pl[:], lhsT=w4[:], rhs=ht[:],
                                 start=True, stop=True)
                lr = scr.tile([1, TW], F32, tag="lgrow", bufs=4)
                nc.vector.tensor_copy(lr[:], pl[:])
                nc.sync.dma_start(lgd[:, n * TW:(n + 1) * TW], lr[:])

        if stages < 5:
            _dummy_outputs(nc, tiny, nrm_out, beta_out, wgt_out)
            return

        # =====================================================
        # Tail: weights = 0.01 + sigmoid(logits + b4) ; WLS fit
        # =====================================================
        with nc.named_scope("tail"):
            lg = scr.tile([BP, N], F32, tag="lg")
            nc.sync.dma_start(lg[:], lgd[:].rearrange("p (b n) -> (p b) n", b=BP))
            w = scr.tile([BP, N], F32, tag="w")
            nc.scalar.activation(w[:], lg[:], AF.Sigmoid, bias=b4[:], scale=1.0)
            nc.vector.tensor_scalar_add(w[:], w[:], 0.01)
            nc.sync.dma_start(wgt_out[:], w[:])

            if stages < 6:
                d3 = tiny.tile([BP, 3], F32, tag="dump3")
                nc.vector.memset(d3[:], 0.0)
                nc.sync.dma_start(nrm_out[:], d3[:])
                nc.sync.dma_start(beta_out[:], d3[:])
                return

            px = scr.tile([BP, N], F32, tag="px")
            py = scr.tile([BP, N], F32, tag="py")
            pz_ = scr.tile([BP, N], F32, tag="pz")
            nc.sync.dma_start(px[:], pts_in[:, 0, :])
            nc.sync.dma_start(py[:], pts_in[:, 1, :])
            nc.sync.dma_start(pz_[:], pts_in[:, 2, :])

            wx = scr.tile([BP, N], F32, tag="wx")
            wy = scr.tile([BP, N], F32, tag="wy")
            wz = scr.tile([BP, N], F32, tag="wz")
            nc.vector.tensor_mul(wx[:], w[:], px[:])
            nc.vector.tensor_mul(wy[:], w[:], py[:])
            nc.vector.tensor_mul(wz[:], w[:], pz_[:])

            def ttr(in0, in1, name):
                o = scr.tile([BP, N], F32, tag="ttr_scr", bufs=4)
                acc = tiny.tile([BP, 1], F32, tag=name)
                nc.vector.tensor_mul(o[:], in0[:], in1[:])
                nc.vector.reduce_sum(acc[:], o[:], axis=AX.X)
                return acc

            def tred(in0, name):
                acc = tiny.tile([BP, 1], F32, tag=name)
                nc.vector.reduce_sum(acc[:], in0[:], axis=AX.X)
                return acc

            if stages < 7:
                d3 = tiny.tile([BP, 3], F32, tag="dump3")
                nc.vector.memset(d3[:], 0.0)
                nc.sync.dma_start(nrm_out[:], d3[:])
                nc.sync.dma_start(beta_out[:], d3[:])
                return

            sxx = ttr(wx, px, "sxx")
            sxy = ttr(wx, py, "sxy")
            syy = ttr(wy, py, "syy")
            sxz = ttr(wx, pz_, "sxz")
            syz = ttr(wy, pz_, "syz")
            sx = tred(wx, "sx")
            sy = tred(wy, "sy")
            sz = tred(wz, "sz")
            sw = tred(w, "sw")

            def tt(op, a, b_, name):
                o = tiny.tile([BP, 1], F32, tag=name)
                nc.vector.tensor_tensor(o[:], a[:], b_[:], op)
                return o

            def fma_sub(a, b_, c, d, name):
                # returns a*b_ - c*d
                t0 = tt(ALU.mult, a, b_, name + "_t0")
                t1 = tt(ALU.mult, c, d, name + "_t1")
                return tt(ALU.subtract, t0, t1, name)

            c00 = fma_sub(syy, sw, sy, sy, "c00")
            c01 = fma_sub(sy, sx, sxy, sw, "c01")
            c02 = fma_sub(sxy, sy, syy, sx, "c02")
            c11 = fma_sub(sxx, sw, sx, sx, "c11")
            c12 = fma_sub(sx, sxy, sxx, sy, "c12")
            c22 = fma_sub(sxx, syy, sxy, sxy, "c22")

            def dot3(a0, b0, a1, b1, a2, b2, name):
                t0 = tt(ALU.mult, a0, b0, name + "_d0")
                t1 = tt(ALU.mult, a1, b1, name + "_d1")
                t2 = tt(ALU.mult, a2, b2, name + "_d2")
                s01 = tt(ALU.add, t0, t1, name + "_s")
                return tt(ALU.add, s01, t2, name)

            if stages < 8:
                d3 = tiny.tile([BP, 3], F32, tag="dump3")
                nc.vector.memset(d3[:], 0.0)
                nc.sync.dma_start(nrm_out[:], d3[:])
                nc.sync.dma_start(beta_out[:], d3[:])
                return

            det = dot3(sxx, c00, sxy, c01, sx, c02, "det")
            rdet = tiny.tile([BP, 1], F32, tag="rdet")
            nc.vector.reciprocal(rdet[:], det[:])

            nb0 = dot3(c00, sxz, c01, syz, c02, sz, "nb0")
            nb1 = dot3(c01, sxz, c11, syz, c12, sz, "nb1")
            nb2 = dot3(c02, sxz, c12, syz, c22, sz, "nb2")
            bt0 = tt(ALU.mult, nb0, rdet, "bt0")
            bt1 = tt(ALU.mult, nb1, rdet, "bt1")
            bt2 = tt(ALU.mult, nb2, rdet, "bt2")

            # normal = [-b0, -b1, 1] / sqrt(b0^2 + b1^2 + 1)
            q0 = tt(ALU.mult, bt0, bt0, "q0")
            q1 = tt(ALU.mult, bt1, bt1, "q1")
            qs = tt(ALU.add, q0, q1, "qs")
            one_t = tiny.tile([BP, 1], F32, tag="one")
            nc.vector.memset(one_t[:], 1.0)
            nrm2 = tt(ALU.add, qs, one_t, "nrm2")
            nrm = tiny.tile([BP, 1], F32, tag="nrm")
            nc.scalar.sqrt(nrm[:], nrm2[:])
            rinv = tiny.tile([BP, 1], F32, tag="rinv")
            nc.vector.reciprocal(rinv[:], nrm[:])

            beta_t = tiny.tile([BP, 3], F32, tag="beta_t")
            nc.vector.tensor_copy(beta_t[:, 0:1], bt0[:])
            nc.vector.tensor_copy(beta_t[:, 1:2], bt1[:])
            nc.vector.tensor_copy(beta_t[:, 2:3], bt2[:])
            nc.sync.dma_start(beta_out[:], beta_t[:])

            nrm_t = tiny.tile([BP, 3], F32, tag="nrm_t")
            neg_rinv = tiny.tile([BP, 1], F32, tag="neg_rinv")
            nc.vector.tensor_scalar_mul(neg_rinv[:], rinv[:], -1.0)
            nc.vector.tensor_mul(nrm_t[:, 0:1], bt0[:], neg_rinv[:])
            nc.vector.tensor_mul(nrm_t[:, 1:2], bt1[:], neg_rinv[:])
            nc.vector.tensor_copy(nrm_t[:, 2:3], rinv[:])
            nc.sync.dma_start(nrm_out[:], nrm_t[:])


def _dummy_outputs(nc, tiny, nrm_out, beta_out, wgt_out):
    d3 = tiny.tile([BP, 3], mybir.dt.float32, tag="dump3")
    nc.vector.memset(d3[:], 0.0)
    nc.sync.dma_start(nrm_out[:], d3[:])
    nc.sync.dma_start(beta_out[:], d3[:])
    dn = tiny.tile([BP, N], mybir.dt.float32, tag="dumpn")
    nc.vector.memset(dn[:], 0.0)
    nc.sync.dma_start(wgt_out[:], dn[:])


def _bn_params(nc, tc, tiny, scr, dram, st_list, g_t, be_t, eps_t, name):
    """Aggregate per-m-chunk bn_stats, AllReduce across cores, and produce
    per-channel scale [128, M] and bias [128, M] tiles for the BN affine."""
    M = len(st_list)
    pay = tiny.tile([128, 2 * M], F32, tag=f"{name}_pay")
    for m in range(M):
        agg = tiny.tile([128, 2], F32, tag=f"{name}_agg{m}")
        nc.vector.bn_aggr(agg[:], st_list[m][:])
        # payload: [mean/8, (var + mean^2)/8]
        nc.vector.tensor_scalar_mul(pay[:, 2 * m:2 * m + 1], agg[:, 0:1], 1.0 / NCORES)
        msq = tiny.tile([128, 1], F32, tag=f"{name}_msq{m}")
        nc.vector.tensor_mul(msq[:], agg[:, 0:1], agg[:, 0:1])
        ez2 = tiny.tile([128, 1], F32, tag=f"{name}_ez2{m}")
        nc.vector.tensor_add(ez2[:], agg[:, 1:2], msq[:])
        nc.vector.tensor_scalar_mul(pay[:, 2 * m + 1:2 * m + 2], ez2[:], 1.0 / NCORES)

    cin = dram.tile([128, 2 * M], F32, name=f"{name}_cin")
    cout = dram.tile([128, 2 * M], F32, name=f"{name}_cout")
    nc.sync.dma_start(cin[:], pay[:])
    nc.gpsimd.collective_compute(
        "AllReduce", ALU.add,
        replica_groups=[list(range(NCORES))],
        ins=[cin.opt()], outs=[cout.opt()])
    ar = tiny.tile([128, 2 * M], F32, tag=f"{name}_ar")
    nc.sync.dma_start(ar[:], cout[:])

    sc = tiny.tile([128, M], F32, tag=f"{name}_sc")
    bi = tiny.tile([128, M], F32, tag=f"{name}_bi")
    for m in range(M):
        mean = ar[:, 2 * m:2 * m + 1]
        ez2 = ar[:, 2 * m + 1:2 * m + 2]
        msq = tiny.tile([128, 1], F32, tag=f"{name}_gm{m}")
        nc.vector.tensor_mul(msq[:], mean, mean)
        var = tiny.tile([128, 1], F32, tag=f"{name}_var{m}")
        nc.vector.tensor_sub(var[:], ez2, msq[:])
        std = tiny.tile([128, 1], F32, tag=f"{name}_std{m}")
        # std = sqrt(var + eps)
        nc.scalar.activation(std[:], var[:], AF.Sqrt, bias=eps_t[:], scale=1.0)
        rstd = tiny.tile([128, 1], F32, tag=f"{name}_rstd{m}")
        nc.vector.reciprocal(rstd[:], std[:])
        nc.vector.tensor_mul(sc[:, m:m + 1], g_t[:, m:m + 1], rstd[:])
        t = tiny.tile([128, 1], F32, tag=f"{name}_t{m}")
        nc.vector.tensor_mul(t[:], sc[:, m:m + 1], mean)
        nc.vector.tensor_sub(bi[:, m:m + 1], be_t[:, m:m + 1], t[:])
    return sc, bi


_NC_CACHE = {}


def _get_nc():
    if "nc" not in _NC_CACHE:
        _NC_CACHE["nc"] = _build()
    return _NC_CACHE["nc"]


def _prepare_in_maps(points, x, w1, w2, w3, w4, g1, be1, g2, be2, g3, be3, b4):
    points = np.asarray(points, dtype=np.float32)
    x = np.asarray(x, dtype=np.float32)
    w1 = np.asarray(w1, dtype=np.float32)
    w2 = np.asarray(w2, dtype=np.float32)
    w3 = np.asarray(w3, dtype=np.float32)
    w4 = np.asarray(w4, dtype=np.float32)
    g1 = np.asarray(g1, dtype=np.float32)
    be1 = np.asarray(be1, dtype=np.float32)
    g2 = np.asarray(g2, dtype=np.float32)
    be2 = np.asarray(be2, dtype=np.float32)
    g3 = np.asarray(g3, dtype=np.float32)
    be3 = np.asarray(be3, dtype=np.float32)
    b4 = np.asarray(b4, dtype=np.float32)
    # NOTE: b1/b2/b3 cancel exactly inside training-mode BatchNorm
    # (y - mean(y) is invariant to a per-channel constant shift), so they
    # are not sent to the device. n_effective_points is unused by the
    # reference module.

    shared = {
        "w1t": np.ascontiguousarray(w1.T),                       # f32r bytes
        "w2t": np.ascontiguousarray(w2.T).astype(np.float16),
        "w3t": np.ascontiguousarray(w3.T).astype(np.float16),
        "w4c": np.ascontiguousarray(w4.reshape(128, 1)).astype(np.float16),
        "g1c": np.ascontiguousarray(g1.reshape(4, 128).T),
        "be1c": np.ascontiguousarray(be1.reshape(4, 128).T),
        "g2c": np.ascontiguousarray(g2.reshape(2, 128).T),
        "be2c": np.ascontiguousarray(be2.reshape(2, 128).T),
        "g3c": np.ascontiguousarray(g3.reshape(1, 128).T),
        "be3c": np.ascontiguousarray(be3.reshape(1, 128).T),
        "b4c": np.full((BP, 1), np.float32(b4.reshape(-1)[0]), dtype=np.float32),
    }
    in_maps = []
    for c in range(NCORES):
        sl = slice(c * BP, (c + 1) * BP)
        m = dict(shared)
        m["x"] = np.ascontiguousarray(x[sl])
        m["points"] = np.ascontiguousarray(points[sl])
        in_maps.append(m)
    return in_maps


def _gather(res):
    normal = np.concatenate([res.results[c]["normal"] for c in range(NCORES)], axis=0)
    beta = np.concatenate([res.results[c]["beta"] for c in range(NCORES)], axis=0)
    weights = np.concatenate([res.results[c]["weights"] for c in range(NCORES)], axis=0)
    return normal.astype(np.float32), beta.astype(np.float32), weights.astype(np.float32)


def kernel(points, x, n_effective_points,
           w1, b1, g1, be1, w2, b2, g2, be2, w3, b3, g3, be3, w4, b4,
           **_unused):
    nc = _get_nc()
    in_maps = _prepare_in_maps(points, x, w1, w2, w3, w4,
                               g1, be1, g2, be2, g3, be3, b4)
    res = run_bass_kernel_spmd(nc, in_maps, core_ids=list(range(NCORES)))
    _NC_CACHE["last_results"] = res
    return _gather(res)


# revision 16
# speedup vs baseline: 1.2711x; 1.2711x over previous
"""Trainium2 Bass kernel for the conv-BN-relu x3 + conv1x1 + sigmoid-weighted
3x3 WLS jet fit module.

Contract: kernel(**inputs) takes the FULL unsharded inputs (numpy arrays,
keys as in reference.setup_inputs()) and returns the FULL output tuple
(normal (512,3), beta (512,3), weights (512,256)), all float32.

Internally: data-parallel over the batch axis B=512 across 8 NeuronCores
(64 neighborhoods each). Training-mode BatchNorm statistics are combined
across cores with a per-layer AllReduce of per-channel [mean, E[z^2]].
"""
import sys
import os

sys.path.insert(0, '/opt/trn_rl_repo')

import numpy as np

import concourse.bass as bass
import concourse.mybir as mybir
import concourse.tile as tile
from concourse import bacc
from concourse.bass_utils import run_bass_kernel_spmd

F32 = mybir.dt.float32
F32R = mybir.dt.float32r
FP16 = mybir.dt.float16
AF = mybir.ActivationFunctionType
ALU = mybir.AluOpType
AX = mybir.AxisListType

NCORES = 8
B, C, N = 512, 1024, 256
BP = B // NCORES            # 64 neighborhoods per core
BN = BP * N                 # 16384 samples per core
TW = 512                    # matmul free-dim tile width
WT = 1024                   # wide tile width for ACT applies / z IO
NT = BN // TW               # 32 matmul n-tiles per core
NW = BN // WT               # 16 wide tiles per core
BPT = TW // N               # 2 neighborhoods per matmul n-tile
EPS_BN = 1e-5


def _build(stages=99):
    nc = bacc.Bacc("TRN2", target_bir_lowering=False, debug=False,
                   enable_asserts=True, num_devices=NCORES)

    # ---- kernel I/O ----
    x_in = nc.dram_tensor("x", (BP, C, N), F32R, kind="ExternalInput").ap()
    pts_in = nc.dram_tensor("points", (BP, 3, N), F32, kind="ExternalInput").ap()
    w1t_in = nc.dram_tensor("w1t", (1024, 512), F32R, kind="ExternalInput").ap()
    w2t_in = nc.dram_tensor("w2t", (512, 256), FP16, kind="ExternalInput").ap()
    w3t_in = nc.dram_tensor("w3t", (256, 128), FP16, kind="ExternalInput").ap()
    w4c_in = nc.dram_tensor("w4c", (128, 1), FP16, kind="ExternalInput").ap()
    g1_in = nc.dram_tensor("g1c", (128, 4), F32, kind="ExternalInput").ap()
    be1_in = nc.dram_tensor("be1c", (128, 4), F32, kind="ExternalInput").ap()
    g2_in = nc.dram_tensor("g2c", (128, 2), F32, kind="ExternalInput").ap()
    be2_in = nc.dram_tensor("be2c", (128, 2), F32, kind="ExternalInput").ap()
    g3_in = nc.dram_tensor("g3c", (128, 1), F32, kind="ExternalInput").ap()
    be3_in = nc.dram_tensor("be3c", (128, 1), F32, kind="ExternalInput").ap()
    b4_in = nc.dram_tensor("b4c", (BP, 1), F32, kind="ExternalInput").ap()

    nrm_out = nc.dram_tensor("normal", (BP, 3), F32, kind="ExternalOutput").ap()
    beta_out = nc.dram_tensor("beta", (BP, 3), F32, kind="ExternalOutput").ap()
    wgt_out = nc.dram_tensor("weights", (BP, N), F32, kind="ExternalOutput").ap()

    with tile.TileContext(nc) as tc:
        _emit(nc, tc, x_in, pts_in,
              w1t_in, w2t_in, w3t_in, w4c_in,
              g1_in, be1_in, g2_in, be2_in, g3_in, be3_in, b4_in,
              nrm_out, beta_out, wgt_out, stages)
    nc.compile()
    return nc


def _emit(nc, tc, x_in, pts_in, w1t_in, w2t_in, w3t_in, w4c_in,
          g1_in, be1_in, g2_in, be2_in, g3_in, be3_in, b4_in,
          nrm_out, beta_out, wgt_out, stages=99):
    from contextlib import ExitStack
    ctx = ExitStack()
    with ctx:
        wpool = ctx.enter_context(tc.tile_pool(name="wpool", bufs=1))
        xpool = ctx.enter_context(tc.tile_pool(name="xpool", bufs=16))
        zevict = ctx.enter_context(tc.tile_pool(name="zevict", bufs=6))
        zload = ctx.enter_context(tc.tile_pool(name="zload", bufs=6))
        hpool = ctx.enter_context(tc.tile_pool(name="hpool", bufs=6))
        spool = ctx.enter_context(tc.tile_pool(name="spool", bufs=1))
        tiny = ctx.enter_context(tc.tile_pool(name="tiny", bufs=1))
        scr = ctx.enter_context(tc.tile_pool(name="scr", bufs=1))
        psum = ctx.enter_context(tc.tile_pool(name="psum", bufs=8, space="PSUM"))
        dram = ctx.enter_context(tc.tile_pool(name="dram", bufs=1, space="DRAM"))

        # ---------------- weights / params load ----------------
        w1 = []
        for k in range(8):
            t = wpool.tile([128, 512], F32R, tag=f"w1_{k}")
            nc.sync.dma_start(t[:], w1t_in[k * 128:(k + 1) * 128, :])
            w1.append(t)
        w2 = []
        for k in range(4):
            t = wpool.tile([128, 256], FP16, tag=f"w2_{k}")
            nc.sync.dma_start(t[:], w2t_in[k * 128:(k + 1) * 128, :])
            w2.append(t)
        w3 = []
        for k in range(2):
            t = wpool.tile([128, 128], FP16, tag=f"w3_{k}")
            nc.sync.dma_start(t[:], w3t_in[k * 128:(k + 1) * 128, :])
            w3.append(t)
        w4 = wpool.tile([128, 1], FP16, tag="w4")
        nc.sync.dma_start(w4[:], w4c_in[:])

        g1 = wpool.tile([128, 4], F32, tag="g1")
        nc.sync.dma_start(g1[:], g1_in[:])
        be1 = wpool.tile([128, 4], F32, tag="be1")
        nc.sync.dma_start(be1[:], be1_in[:])
        g2 = wpool.tile([128, 2], F32, tag="g2")
        nc.sync.dma_start(g2[:], g2_in[:])
        be2 = wpool.tile([128, 2], F32, tag="be2")
        nc.sync.dma_start(be2[:], be2_in[:])
        g3 = wpool.tile([128, 1], F32, tag="g3")
        nc.sync.dma_start(g3[:], g3_in[:])
        be3 = wpool.tile([128, 1], F32, tag="be3")
        nc.sync.dma_start(be3[:], be3_in[:])
        b4 = wpool.tile([BP, 1], F32, tag="b4")
        nc.sync.dma_start(b4[:], b4_in[:])

        eps_t = tiny.tile([128, 1], F32, tag="eps")
        nc.vector.memset(eps_t[:], EPS_BN)

        # Preload the sigmoid ACT table set early (it contains relu/copy as
        # filler, so the phase applies don't trigger any further table load;
        # only the tail's sqrt swaps sets once).
        warm = tiny.tile([1, 1], F32, tag="warm")
        nc.vector.memset(warm[:], 0.0)
        warm2 = tiny.tile([1, 1], F32, tag="warm2")
        nc.scalar.activation(warm2[:], warm[:], AF.Sigmoid)

        # ---------------- DRAM scratch for z (fp16) ----------------
        z1d = [dram.tile([128, BN], FP16, name=f"z1d_{m}") for m in range(4)]
        z2d = [dram.tile([128, BN], FP16, name=f"z2d_{m}") for m in range(2)]
        z3d = [dram.tile([128, BN], FP16, name=f"z3d_{m}") for m in range(1)]
        lgd = dram.tile([1, BN], F32)

        # per-layer bn_stats collection tiles: [128, NT*6] per m-chunk
        st1 = [spool.tile([128, NT * 6], F32, name=f"st1_{m}") for m in range(4)]
        st2 = [spool.tile([128, NT * 6], F32, name=f"st2_{m}") for m in range(2)]
        st3 = [spool.tile([128, NT * 6], F32, name=f"st3_{m}") for m in range(1)]

        # =====================================================
        # Phase A1: z1 = W1 @ x ; bn stats ; store z1 (fp16)
        # PE: matmuls; DVE: psum->fp16 eviction + bn_stats;
        # sync queue: x loads; gpsimd queue: z1 stores.
        # =====================================================
        with nc.named_scope("phaseA1"):
            for n in range(NT):
                b0 = n * BPT
                xts = []
                for k in range(8):
                    xt = xpool.tile([128, BPT, N], F32R, tag="xt")
                    src = x_in[b0:b0 + BPT, k * 128:(k + 1) * 128, :]
                    nc.sync.dma_start(xt[:], src.rearrange("b c n -> c b n"))
                    xts.append(xt)
                for m in range(4):
                    pz = psum.tile([128, TW], F32, tag="ps")
                    for k in range(8):
                        nc.tensor.matmul(
                            pz[:],
                            lhsT=w1[k][:, m * 128:(m + 1) * 128],
                            rhs=xts[k][:].rearrange("c b n -> c (b n)"),
                            start=(k == 0), stop=(k == 7))
                    zsb = zevict.tile([128, TW], FP16, tag="zsb")
                    nc.vector.tensor_copy(zsb[:], pz[:])
                    nc.gpsimd.dma_start(z1d[m][:, n * TW:(n + 1) * TW], zsb[:])
                    nc.vector.bn_stats(st1[m][:, n * 6:(n + 1) * 6], pz[:])

        if stages < 2:
            _dummy_outputs(nc, tiny, nrm_out, beta_out, wgt_out)
            return
        sc1, bi1 = _bn_params(nc, tc, tiny, dram, st1, g1, be1, eps_t, "bn1")

        # =====================================================
        # Phase B1: h1 = relu(bn(z1)) (wide 1024 ACT) ; z2 = W2 @ h1
        # =====================================================
        with nc.named_scope("phaseB1"):
            for nw in range(NW):
                hts = []
                for k in range(4):
                    zt = zload.tile([128, WT], FP16, tag="z1ld")
                    nc.gpsimd.dma_start(zt[:], z1d[k][:, nw * WT:(nw + 1) * WT])
                    ht = hpool.tile([128, WT], FP16, tag="h1")
                    nc.scalar.activation(ht[:], zt[:], AF.Relu,
                                         bias=bi1[:, k:k + 1], scale=sc1[:, k:k + 1])
                    hts.append(ht)
                for half in range(2):
                    n = nw * 2 + half
                    hsl = slice(half * TW, (half + 1) * TW)
                    for m in range(2):
                        pz = psum.tile([128, TW], F32, tag="ps")
                        for k in range(4):
                            nc.tensor.matmul(
                                pz[:],
                                lhsT=w2[k][:, m * 128:(m + 1) * 128],
                                rhs=hts[k][:, hsl],
                                start=(k == 0), stop=(k == 3))
                        zsb = zevict.tile([128, TW], FP16, tag="zsb")
                        nc.vector.tensor_copy(zsb[:], pz[:])
                        nc.gpsimd.dma_start(z2d[m][:, n * TW:(n + 1) * TW], zsb[:])
                        nc.vector.bn_stats(st2[m][:, n * 6:(n + 1) * 6], pz[:])

        if stages < 3:
            _dummy_outputs(nc, tiny, nrm_out, beta_out, wgt_out)
            return
        sc2, bi2 = _bn_params(nc, tc, tiny, dram, st2, g2, be2, eps_t, "bn2")

        # =====================================================
        # Phase B2: h2 = relu(bn(z2)) ; z3 = W3 @ h2
        # =====================================================
        with nc.named_scope("phaseB2"):
            for nw in range(NW):
                hts = []
                for k in range(2):
                    zt = zload.tile([128, WT], FP16, tag="z2ld")
                    nc.gpsimd.dma_start(zt[:], z2d[k][:, nw * WT:(nw + 1) * WT])
                    ht = hpool.tile([128, WT], FP16, tag="h2")
                    nc.scalar.activation(ht[:], zt[:], AF.Relu,
                                         bias=bi2[:, k:k + 1], scale=sc2[:, k:k + 1])
                    hts.append(ht)
                for half in range(2):
                    n = nw * 2 + half
                    hsl = slice(half * TW, (half + 1) * TW)
                    pz = psum.tile([128, TW], F32, tag="ps")
                    for k in range(2):
                        nc.tensor.matmul(
                            pz[:], lhsT=w3[k][:], rhs=hts[k][:, hsl],
                            start=(k == 0), stop=(k == 1))
                    zsb = zevict.tile([128, TW], FP16, tag="zsb")
                    nc.vector.tensor_copy(zsb[:], pz[:])
                    nc.gpsimd.dma_start(z3d[0][:, n * TW:(n + 1) * TW], zsb[:])
                    nc.vector.bn_stats(st3[0][:, n * 6:(n + 1) * 6], pz[:])

        if stages < 4:
            _dummy_outputs(nc, tiny, nrm_out, beta_out, wgt_out)
            return
        sc3, bi3 = _bn_params(nc, tc, tiny, dram, st3, g3, be3, eps_t, "bn3")

        # =====================================================
        # Phase B3: h3 = relu(bn(z3)) ; logits = w4 @ h3 -> DRAM row
        # =====================================================
        with nc.named_scope("phaseB3"):
            for nw in range(NW):
                zt = zload.tile([128, WT], FP16, tag="z3ld")
                nc.gpsimd.dma_start(zt[:], z3d[0][:, nw * WT:(nw + 1) * WT])
                ht = hpool.tile([128, WT], FP16, tag="h3")
                nc.scalar.activation(ht[:], zt[:], AF.Relu,
                                     bias=bi3[:, 0:1], scale=sc3[:, 0:1])
                for half in range(2):
                    n = nw * 2 + half
                    hsl = slice(half * TW, (half + 1) * TW)
                    pl = psum.tile([1, TW], F32, tag="ps")
                    nc.tensor.matmul(pl[:], lhsT=w4[:], rhs=ht[:, hsl],
                                     start=True, stop=True)
                    lr = scr.tile([1, TW], F32, tag="lgrow", bufs=4)
                    nc.vector.tensor_copy(lr[:], pl[:])
                    nc.sync.dma_start(lgd[:, n * TW:(n + 1) * TW], lr[:])

        if stages < 5:
            _dummy_outputs(nc, tiny, nrm_out, beta_out, wgt_out)
            return

        # =====================================================
        # Tail: weights = 0.01 + sigmoid(logits + b4) ; WLS fit.
        # Everything on DVE except sigmoid/sqrt (ACT), minimizing
        # cross-engine dependency latency.
        # =====================================================
        with nc.named_scope("tail"):
            lg = scr.tile([BP, N], F32, tag="lg")
            nc.sync.dma_start(lg[:], lgd[:].rearrange("p (b n) -> (p b) n", b=BP))
            wsg = scr.tile([BP, N], F32, tag="wsg")
            nc.scalar.activation(wsg[:], lg[:], AF.Sigmoid, bias=b4[:], scale=1.0)
            w = scr.tile([BP, N], F32, tag="w")
            nc.vector.tensor_scalar_add(w[:], wsg[:], 0.01)
            nc.sync.dma_start(wgt_out[:], w[:])

            px = scr.tile([BP, N], F32, tag="px")
            py = scr.tile([BP, N], F32, tag="py")
            pz_ = scr.tile([BP, N], F32, tag="pz")
            nc.sync.dma_start(px[:], pts_in[:, 0, :])
            nc.sync.dma_start(py[:], pts_in[:, 1, :])
            nc.sync.dma_start(pz_[:], pts_in[:, 2, :])

            wx = scr.tile([BP, N], F32, tag="wx")
            wy = scr.tile([BP, N], F32, tag="wy")
            wz = scr.tile([BP, N], F32, tag="wz")
            nc.vector.tensor_mul(wx[:], w[:], px[:])
            nc.vector.tensor_mul(wy[:], w[:], py[:])
            nc.vector.tensor_mul(wz[:], w[:], pz_[:])

            def fused_dot(in0, in1, name):
                # accum_out = sum(in0 * in1) in a single DVE op
                o = scr.tile([BP, N], F32, tag="ttr_scr", bufs=4)
                acc = tiny.tile([BP, 1], F32, tag=name)
                nc.vector.scalar_tensor_tensor(
                    o[:], in0=in0[:], scalar=1.0, in1=in1[:],
                    op0=ALU.mult, op1=ALU.mult, accum_out=acc[:])
                return acc

            def tred(in0, name):
                acc = tiny.tile([BP, 1], F32, tag=name)
                nc.vector.reduce_sum(acc[:], in0[:], axis=AX.X)
                return acc

            sxx = fused_dot(wx, px, "sxx")
            sxy = fused_dot(wx, py, "sxy")
            syy = fused_dot(wy, py, "syy")
            sxz = fused_dot(wx, pz_, "sxz")
            syz = fused_dot(wy, pz_, "syz")
            sx = tred(wx, "sx")
            sy = tred(wy, "sy")
            sz = tred(wz, "sz")
            sw = tred(w, "sw")

            def tt(op, a, b_, name):
                o = tiny.tile([BP, 1], F32, tag=name)
                nc.vector.tensor_tensor(o[:], a[:], b_[:], op)
                return o

            def fms(a, bs, t, name):
                # a * bs - t   (bs used as per-partition scalar operand)
                o = tiny.tile([BP, 1], F32, tag=name)
                nc.vector.scalar_tensor_tensor(
                    o[:], in0=a[:], scalar=bs[:], in1=t[:],
                    op0=ALU.mult, op1=ALU.subtract)
                return o

            def fma(a, bs, t, name):
                # a * bs + t
                o = tiny.tile([BP, 1], F32, tag=name)
                nc.vector.scalar_tensor_tensor(
                    o[:], in0=a[:], scalar=bs[:], in1=t[:],
                    op0=ALU.mult, op1=ALU.add)
                return o

            t_yy = tt(ALU.mult, sy, sy, "t_yy")
            c00 = fms(syy, sw, t_yy, "c00")
            t_xyw = tt(ALU.mult, sxy, sw, "t_xyw")
            c01 = fms(sy, sx, t_xyw, "c01")
            t_yyx = tt(ALU.mult, syy, sx, "t_yyx")
            c02 = fms(sxy, sy, t_yyx, "c02")
            t_xx = tt(ALU.mult, sx, sx, "t_xx")
            c11 = fms(sxx, sw, t_xx, "c11")
            t_xxy = tt(ALU.mult, sxx, sy, "t_xxy")
            c12 = fms(sx, sxy, t_xxy, "c12")
            t_xy2 = tt(ALU.mult, sxy, sxy, "t_xy2")
            c22 = fms(sxx, syy, t_xy2, "c22")

            def dot3(a0, b0, a1, b1, a2, b2, name):
                t1 = tt(ALU.mult, a1, b1, name + "_t1")
                t01 = fma(a0, b0, t1, name + "_t01")
                return fma(a2, b2, t01, name)

            det = dot3(sxx, c00, sxy, c01, sx, c02, "det")
            rdet = tiny.tile([BP, 1], F32, tag="rdet")
            nc.vector.reciprocal(rdet[:], det[:])

            nb0 = dot3(c00, sxz, c01, syz, c02, sz, "nb0")
            nb1 = dot3(c01, sxz, c11, syz, c12, sz, "nb1")
            nb2 = dot3(c02, sxz, c12, syz, c22, sz, "nb2")
            bt0 = tt(ALU.mult, nb0, rdet, "bt0")
            bt1 = tt(ALU.mult, nb1, rdet, "bt1")
            bt2 = tt(ALU.mult, nb2, rdet, "bt2")

            beta_t = tiny.tile([BP, 3], F32, tag="beta_t")
            nc.vector.tensor_copy(beta_t[:, 0:1], bt0[:])
            nc.vector.tensor_copy(beta_t[:, 1:2], bt1[:])
            nc.vector.tensor_copy(beta_t[:, 2:3], bt2[:])
            nc.sync.dma_start(beta_out[:], beta_t[:])

            # normal = [-b0, -b1, 1] / sqrt(b0^2 + b1^2 + 1)
            q1 = tt(ALU.mult, bt1, bt1, "q1")
            qs = fma(bt0, bt0, q1, "qs")
            nrm2 = tiny.tile([BP, 1], F32, tag="nrm2")
            nc.vector.tensor_scalar_add(nrm2[:], qs[:], 1.0)
            nrm = tiny.tile([BP, 1], F32, tag="nrm")
            nc.scalar.sqrt(nrm[:], nrm2[:])
            rinv = tiny.tile([BP, 1], F32, tag="rinv")
            nc.vector.reciprocal(rinv[:], nrm[:])
            neg_rinv = tiny.tile([BP, 1], F32, tag="neg_rinv")
            nc.vector.tensor_scalar_mul(neg_rinv[:], rinv[:], -1.0)

            nrm_t = tiny.tile([BP, 3], F32, tag="nrm_t")
            nc.vector.tensor_mul(nrm_t[:, 0:1], bt0[:], neg_rinv[:])
            nc.vector.tensor_mul(nrm_t[:, 1:2], bt1[:], neg_rinv[:])
            nc.vector.tensor_copy(nrm_t[:, 2:3], rinv[:])
            nc.sync.dma_start(nrm_out[:], nrm_t[:])


def _dummy_outputs(nc, tiny, nrm_out, beta_out, wgt_out):
    d3 = tiny.tile([BP, 3], mybir.dt.float32, tag="dump3")
    nc.vector.memset(d3[:], 0.0)
    nc.sync.dma_start(nrm_out[:], d3[:])
    nc.sync.dma_start(beta_out[:], d3[:])
    dn = tiny.tile([BP, N], mybir.dt.float32, tag="dumpn")
    nc.vector.memset(dn[:], 0.0)
    nc.sync.dma_start(wgt_out[:], dn[:])


def _bn_params(nc, tc, tiny, dram, st_list, g_t, be_t, eps_t, name):
    """Aggregate per-m-chunk bn_stats, AllReduce [mean, E[z^2]] across cores,
    produce scale [128, M] / bias [128, M] for the BN affine. Per-channel
    math is vectorized across the M chunks to keep the dependency chain
    short."""
    M = len(st_list)
    agg = tiny.tile([128, 2 * M], F32, tag=f"{name}_agg")
    for m in range(M):
        nc.vector.bn_aggr(agg[:, 2 * m:2 * m + 2], st_list[m][:])
    means = agg[:, 0:2 * M:2]
    varis = agg[:, 1:2 * M:2]
    # payload: [mean/8 ..., (var + mean^2)/8 ...] in [128, 2M]
    pay = tiny.tile([128, 2 * M], F32, tag=f"{name}_pay")
    nc.vector.tensor_scalar_mul(pay[:, 0:M], means, 1.0 / NCORES)
    ez2 = tiny.tile([128, M], F32, tag=f"{name}_ez2")
    nc.vector.tensor_tensor(ez2[:], means, means, ALU.mult)
    nc.vector.tensor_add(ez2[:], ez2[:], varis)
    nc.vector.tensor_scalar_mul(pay[:, M:2 * M], ez2[:], 1.0 / NCORES)

    cin = dram.tile([128, 2 * M], F32, name=f"{name}_cin")
    cout = dram.tile([128, 2 * M], F32, name=f"{name}_cout")
    nc.sync.dma_start(cin[:], pay[:])
    nc.gpsimd.collective_compute(
        "AllReduce", ALU.add,
        replica_groups=[list(range(NCORES))],
        ins=[cin.opt()], outs=[cout.opt()])
    ar = tiny.tile([128, 2 * M], F32, tag=f"{name}_ar")
    nc.sync.dma_start(ar[:], cout[:])

    gmean = ar[:, 0:M]
    gez2 = ar[:, M:2 * M]
    # var = E[z^2] - mean^2 ; rstd = 1/sqrt(var+eps)
    var = tiny.tile([128, M], F32, tag=f"{name}_var")
    nc.vector.tensor_tensor(var[:], gmean, gmean, ALU.mult)
    nc.vector.tensor_sub(var[:], gez2, var[:])
    std = tiny.tile([128, M], F32, tag=f"{name}_std")
    nc.scalar.activation(std[:], var[:], AF.Sqrt, bias=eps_t[:], scale=1.0)
    rstd = tiny.tile([128, M], F32, tag=f"{name}_rstd")
    nc.vector.reciprocal(rstd[:], std[:])
    sc = tiny.tile([128, M], F32, tag=f"{name}_sc")
    nc.vector.tensor_mul(sc[:], g_t[:, 0:M], rstd[:])
    bi = tiny.tile([128, M], F32, tag=f"{name}_bi")
    nc.vector.tensor_mul(bi[:], sc[:], gmean)
    nc.vector.tensor_sub(bi[:], be_t[:, 0:M], bi[:])
    return sc, bi


_NC_CACHE = {}


def _get_nc():
    if "nc" not in _NC_CACHE:
        _NC_CACHE["nc"] = _build()
    return _NC_CACHE["nc"]


def _prepare_in_maps(points, x, w1, w2, w3, w4, g1, be1, g2, be2, g3, be3, b4):
    points = np.asarray(points, dtype=np.float32)
    x = np.asarray(x, dtype=np.float32)
    w1 = np.asarray(w1, dtype=np.float32)
    w2 = np.asarray(w2, dtype=np.float32)
    w3 = np.asarray(w3, dtype=np.float32)
    w4 = np.asarray(w4, dtype=np.float32)
    g1 = np.asarray(g1, dtype=np.float32)
    be1 = np.asarray(be1, dtype=np.float32)
    g2 = np.asarray(g2, dtype=np.float32)
    be2 = np.asarray(be2, dtype=np.float32)
    g3 = np.asarray(g3, dtype=np.float32)
    be3 = np.asarray(be3, dtype=np.float32)
    b4 = np.asarray(b4, dtype=np.float32)
    # NOTE: b1/b2/b3 cancel exactly inside training-mode BatchNorm
    # (y - mean(y) is invariant to a per-channel constant shift), so they
    # are not sent to the device. n_effective_points is unused by the
    # reference module.

    shared = {
        "w1t": np.ascontiguousarray(w1.T),                       # f32r bytes
        "w2t": np.ascontiguousarray(w2.T).astype(np.float16),
        "w3t": np.ascontiguousarray(w3.T).astype(np.float16),
        "w4c": np.ascontiguousarray(w4.reshape(128, 1)).astype(np.float16),
        "g1c": np.ascontiguousarray(g1.reshape(4, 128).T),
        "be1c": np.ascontiguousarray(be1.reshape(4, 128).T),
        "g2c": np.ascontiguousarray(g2.reshape(2, 128).T),
        "be2c": np.ascontiguousarray(be2.reshape(2, 128).T),
        "g3c": np.ascontiguousarray(g3.reshape(1, 128).T),
        "be3c": np.ascontiguousarray(be3.reshape(1, 128).T),
        "b4c": np.full((BP, 1), np.float32(b4.reshape(-1)[0]), dtype=np.float32),
    }
    in_maps = []
    for c in range(NCORES):
        sl = slice(c * BP, (c + 1) * BP)
        m = dict(shared)
        m["x"] = np.ascontiguousarray(x[sl])
        m["points"] = np.ascontiguousarray(points[sl])
        in_maps.append(m)
    return in_maps


def _gather(res):
    normal = np.concatenate([res.results[c]["normal"] for c in range(NCORES)], axis=0)
    beta = np.concatenate([res.results[c]["beta"] for c in range(NCORES)], axis=0)
    weights = np.concatenate([res.results[c]["weights"] for c in range(NCORES)], axis=0)
    return normal.astype(np.float32), beta.astype(np.float32), weights.astype(np.float32)


def kernel(points, x, n_effective_points,
           w1, b1, g1, be1, w2, b2, g2, be2, w3, b3, g3, be3, w4, b4,
           **_unused):
    nc = _get_nc()
    in_maps = _prepare_in_maps(points, x, w1, w2, w3, w4,
                               g1, be1, g2, be2, g3, be3, b4)
    res = run_bass_kernel_spmd(nc, in_maps, core_ids=list(range(NCORES)))
    _NC_CACHE["last_results"] = res
    return _gather(res)


# revision 19
# speedup vs baseline: 1.4399x; 1.1328x over previous
"""Trainium2 Bass kernel for the conv-BN-relu x3 + conv1x1 + sigmoid-weighted
3x3 WLS jet fit module.

Contract: kernel(**inputs) takes the FULL unsharded inputs (numpy arrays,
keys as in reference.setup_inputs()) and returns the FULL output tuple
(normal (512,3), beta (512,3), weights (512,256)), all float32.

Internally: data-parallel over the batch axis B=512 across 8 NeuronCores
(64 neighborhoods each). Training-mode BatchNorm statistics are combined
across cores with a per-layer AllReduce of per-channel [mean, E[z^2]].
"""
import sys
import os

sys.path.insert(0, '/opt/trn_rl_repo')

import numpy as np

import concourse.bass as bass
import concourse.mybir as mybir
import concourse.tile as tile
from concourse import bacc
from concourse.bass_utils import run_bass_kernel_spmd

F32 = mybir.dt.float32
F32R = mybir.dt.float32r
FP16 = mybir.dt.float16
AF = mybir.ActivationFunctionType
ALU = mybir.AluOpType
AX = mybir.AxisListType

NCORES = 8
B, C, N = 512, 1024, 256
BP = B // NCORES            # 64 neighborhoods per core
BN = BP * N                 # 16384 samples per core
TW = 512                    # matmul free-dim tile width
WT = 1024                   # wide tile width for ACT applies / z IO
NT = BN // TW               # 32 matmul n-tiles per core
NW = BN // WT               # 16 wide tiles per core
BPT = TW // N               # 2 neighborhoods per matmul n-tile
EPS_BN = 1e-5


def _build(stages=99):
    nc = bacc.Bacc("TRN2", target_bir_lowering=False, debug=False,
                   enable_asserts=True, num_devices=NCORES)

    # ---- kernel I/O ----
    x_in = nc.dram_tensor("x", (BP, C, N), FP16, kind="ExternalInput").ap()
    pts_in = nc.dram_tensor("points", (BP, 3, N), F32, kind="ExternalInput").ap()
    w1t_in = nc.dram_tensor("w1t", (1024, 512), FP16, kind="ExternalInput").ap()
    w2t_in = nc.dram_tensor("w2t", (512, 256), FP16, kind="ExternalInput").ap()
    w3t_in = nc.dram_tensor("w3t", (256, 128), FP16, kind="ExternalInput").ap()
    w4c_in = nc.dram_tensor("w4c", (128, 1), FP16, kind="ExternalInput").ap()
    g1_in = nc.dram_tensor("g1c", (128, 4), F32, kind="ExternalInput").ap()
    be1_in = nc.dram_tensor("be1c", (128, 4), F32, kind="ExternalInput").ap()
    g2_in = nc.dram_tensor("g2c", (128, 2), F32, kind="ExternalInput").ap()
    be2_in = nc.dram_tensor("be2c", (128, 2), F32, kind="ExternalInput").ap()
    g3_in = nc.dram_tensor("g3c", (128, 1), F32, kind="ExternalInput").ap()
    be3_in = nc.dram_tensor("be3c", (128, 1), F32, kind="ExternalInput").ap()
    b4_in = nc.dram_tensor("b4c", (BP, 1), F32, kind="ExternalInput").ap()

    nrm_out = nc.dram_tensor("normal", (BP, 3), F32, kind="ExternalOutput").ap()
    beta_out = nc.dram_tensor("beta", (BP, 3), F32, kind="ExternalOutput").ap()
    wgt_out = nc.dram_tensor("weights", (BP, N), F32, kind="ExternalOutput").ap()

    with tile.TileContext(nc) as tc:
        _emit(nc, tc, x_in, pts_in,
              w1t_in, w2t_in, w3t_in, w4c_in,
              g1_in, be1_in, g2_in, be2_in, g3_in, be3_in, b4_in,
              nrm_out, beta_out, wgt_out, stages)
    nc.compile()
    return nc


def _emit(nc, tc, x_in, pts_in, w1t_in, w2t_in, w3t_in, w4c_in,
          g1_in, be1_in, g2_in, be2_in, g3_in, be3_in, b4_in,
          nrm_out, beta_out, wgt_out, stages=99):
    from contextlib import ExitStack
    ctx = ExitStack()
    with ctx:
        wpool = ctx.enter_context(tc.tile_pool(name="wpool", bufs=1))
        xpool = ctx.enter_context(tc.tile_pool(name="xpool", bufs=16))
        zevict = ctx.enter_context(tc.tile_pool(name="zevict", bufs=6))
        zload = ctx.enter_context(tc.tile_pool(name="zload", bufs=6))
        hpool = ctx.enter_context(tc.tile_pool(name="hpool", bufs=6))
        spool = ctx.enter_context(tc.tile_pool(name="spool", bufs=1))
        tiny = ctx.enter_context(tc.tile_pool(name="tiny", bufs=1))
        scr = ctx.enter_context(tc.tile_pool(name="scr", bufs=1))
        psum = ctx.enter_context(tc.tile_pool(name="psum", bufs=8, space="PSUM"))
        dram = ctx.enter_context(tc.tile_pool(name="dram", bufs=1, space="DRAM"))

        # ---------------- weights / params load ----------------
        w1 = []
        for k in range(8):
            t = wpool.tile([128, 512], FP16, tag=f"w1_{k}")
            nc.sync.dma_start(t[:], w1t_in[k * 128:(k + 1) * 128, :])
            w1.append(t)
        w2 = []
        for k in range(4):
            t = wpool.tile([128, 256], FP16, tag=f"w2_{k}")
            nc.sync.dma_start(t[:], w2t_in[k * 128:(k + 1) * 128, :])
            w2.append(t)
        w3 = []
        for k in range(2):
            t = wpool.tile([128, 128], FP16, tag=f"w3_{k}")
            nc.sync.dma_start(t[:], w3t_in[k * 128:(k + 1) * 128, :])
            w3.append(t)
        w4 = wpool.tile([128, 1], FP16, tag="w4")
        nc.sync.dma_start(w4[:], w4c_in[:])

        g1 = wpool.tile([128, 4], F32, tag="g1")
        nc.sync.dma_start(g1[:], g1_in[:])
        be1 = wpool.tile([128, 4], F32, tag="be1")
        nc.sync.dma_start(be1[:], be1_in[:])
        g2 = wpool.tile([128, 2], F32, tag="g2")
        nc.sync.dma_start(g2[:], g2_in[:])
        be2 = wpool.tile([128, 2], F32, tag="be2")
        nc.sync.dma_start(be2[:], be2_in[:])
        g3 = wpool.tile([128, 1], F32, tag="g3")
        nc.sync.dma_start(g3[:], g3_in[:])
        be3 = wpool.tile([128, 1], F32, tag="be3")
        nc.sync.dma_start(be3[:], be3_in[:])
        b4 = wpool.tile([BP, 1], F32, tag="b4")
        nc.sync.dma_start(b4[:], b4_in[:])

        eps_t = tiny.tile([128, 1], F32, tag="eps")
        nc.vector.memset(eps_t[:], EPS_BN)

        # Preload the sigmoid ACT table set early (it contains relu/copy as
        # filler, so the phase applies don't trigger any further table load;
        # only the tail's sqrt swaps sets once).
        warm = tiny.tile([1, 1], F32, tag="warm")
        nc.vector.memset(warm[:], 0.0)
        warm2 = tiny.tile([1, 1], F32, tag="warm2")
        nc.scalar.activation(warm2[:], warm[:], AF.Sigmoid)

        # ---------------- DRAM scratch for z (fp16) ----------------
        z1d = [dram.tile([128, BN], FP16, name=f"z1d_{m}") for m in range(4)]
        z2sb = [spool.tile([128, BN], FP16, name=f"z2sb_{m}") for m in range(2)]
        z3sb = [spool.tile([128, BN], FP16, name=f"z3sb_{m}") for m in range(1)]
        lgd = dram.tile([1, BN], F32)

        # per-layer bn_stats collection tiles: [128, NT*6] per m-chunk
        st1 = [spool.tile([128, NT * 6], F32, name=f"st1_{m}") for m in range(4)]
        st2 = [spool.tile([128, NT * 6], F32, name=f"st2_{m}") for m in range(2)]
        st3 = [spool.tile([128, NT * 6], F32, name=f"st3_{m}") for m in range(1)]

        # =====================================================
        # Phase A1: z1 = W1 @ x ; bn stats ; store z1 (fp16)
        # PE: matmuls; DVE: psum->fp16 eviction + bn_stats;
        # sync queue: x loads; gpsimd queue: z1 stores.
        # =====================================================
        with nc.named_scope("phaseA1"):
            for n in range(NT):
                b0 = n * BPT
                xts = []
                for k in range(8):
                    xt = xpool.tile([128, BPT, N], FP16, tag="xt")
                    src = x_in[b0:b0 + BPT, k * 128:(k + 1) * 128, :]
                    nc.sync.dma_start(xt[:], src.rearrange("b c n -> c b n"))
                    xts.append(xt)
                for m in range(4):
                    pz = psum.tile([128, TW], F32, tag="ps")
                    for k in range(8):
                        nc.tensor.matmul(
                            pz[:],
                            lhsT=w1[k][:, m * 128:(m + 1) * 128],
                            rhs=xts[k][:].rearrange("c b n -> c (b n)"),
                            start=(k == 0), stop=(k == 7))
                    zsb = zevict.tile([128, TW], FP16, tag="zsb")
                    nc.scalar.copy(zsb[:], pz[:])
                    nc.gpsimd.dma_start(z1d[m][:, n * TW:(n + 1) * TW], zsb[:])
                    nc.vector.bn_stats(st1[m][:, n * 6:(n + 1) * 6], pz[:])

        if stages < 2:
            _dummy_outputs(nc, tiny, nrm_out, beta_out, wgt_out)
            return
        sc1, bi1 = _bn_params(nc, tc, tiny, dram, st1, g1, be1, eps_t, "bn1")

        # =====================================================
        # Phase B1: h1 = relu(bn(z1)) (wide 1024 ACT) ; z2 = W2 @ h1
        # =====================================================
        with nc.named_scope("phaseB1"):
            for nw in range(NW):
                hts = []
                for k in range(4):
                    zt = zload.tile([128, WT], FP16, tag="z1ld")
                    nc.sync.dma_start(zt[:], z1d[k][:, nw * WT:(nw + 1) * WT])
                    ht = hpool.tile([128, WT], FP16, tag="h1")
                    nc.scalar.activation(ht[:], zt[:], AF.Relu,
                                         bias=bi1[:, k:k + 1], scale=sc1[:, k:k + 1])
                    hts.append(ht)
                for half in range(2):
                    n = nw * 2 + half
                    hsl = slice(half * TW, (half + 1) * TW)
                    for m in range(2):
                        pz = psum.tile([128, TW], F32, tag="ps")
                        for k in range(4):
                            nc.tensor.matmul(
                                pz[:],
                                lhsT=w2[k][:, m * 128:(m + 1) * 128],
                                rhs=hts[k][:, hsl],
                                start=(k == 0), stop=(k == 3))
                        dst = z2sb[m][:, n * TW:(n + 1) * TW]
                        if half == 0 and m == 0:
                            nc.scalar.copy(dst, pz[:])
                        else:
                            nc.vector.tensor_copy(dst, pz[:])
                        nc.vector.bn_stats(st2[m][:, n * 6:(n + 1) * 6], pz[:])

        if stages < 3:
            _dummy_outputs(nc, tiny, nrm_out, beta_out, wgt_out)
            return
        sc2, bi2 = _bn_params(nc, tc, tiny, dram, st2, g2, be2, eps_t, "bn2")

        # =====================================================
        # Phase B2: h2 = relu(bn(z2)) ; z3 = W3 @ h2
        # =====================================================
        with nc.named_scope("phaseB2"):
            for nw in range(NW):
                hts = []
                for k in range(2):
                    ht = hpool.tile([128, WT], FP16, tag="h2")
                    nc.scalar.activation(ht[:], z2sb[k][:, nw * WT:(nw + 1) * WT],
                                         AF.Relu,
                                         bias=bi2[:, k:k + 1], scale=sc2[:, k:k + 1])
                    hts.append(ht)
                for half in range(2):
                    n = nw * 2 + half
                    hsl = slice(half * TW, (half + 1) * TW)
                    pz = psum.tile([128, TW], F32, tag="ps")
                    for k in range(2):
                        nc.tensor.matmul(
                            pz[:], lhsT=w3[k][:], rhs=hts[k][:, hsl],
                            start=(k == 0), stop=(k == 1))
                    nc.vector.tensor_copy(z3sb[0][:, n * TW:(n + 1) * TW], pz[:])
                    nc.vector.bn_stats(st3[0][:, n * 6:(n + 1) * 6], pz[:])

        if stages < 4:
            _dummy_outputs(nc, tiny, nrm_out, beta_out, wgt_out)
            return
        sc3, bi3 = _bn_params(nc, tc, tiny, dram, st3, g3, be3, eps_t, "bn3")

        # =====================================================
        # Phase B3: h3 = relu(bn(z3)) ; logits = w4 @ h3 -> DRAM row
        # =====================================================
        with nc.named_scope("phaseB3"):
            for nw in range(NW):
                ht = hpool.tile([128, WT], FP16, tag="h3")
                nc.scalar.activation(ht[:], z3sb[0][:, nw * WT:(nw + 1) * WT],
                                     AF.Relu,
                                     bias=bi3[:, 0:1], scale=sc3[:, 0:1])
                for half in range(2):
                    n = nw * 2 + half
                    hsl = slice(half * TW, (half + 1) * TW)
                    pl = psum.tile([1, TW], F32, tag="ps")
                    nc.tensor.matmul(pl[:], lhsT=w4[:], rhs=ht[:, hsl],
                                     start=True, stop=True)
                    lr = scr.tile([1, TW], F32, tag="lgrow", bufs=4)
                    nc.vector.tensor_copy(lr[:], pl[:])
                    nc.sync.dma_start(lgd[:, n * TW:(n + 1) * TW], lr[:])

        if stages < 5:
            _dummy_outputs(nc, tiny, nrm_out, beta_out, wgt_out)
            return

        # =====================================================
        # Tail: weights = 0.01 + sigmoid(logits + b4) ; WLS fit.
        # Everything on DVE except sigmoid/sqrt (ACT), minimizing
        # cross-engine dependency latency.
        # =====================================================
        with nc.named_scope("tail"):
            lg = scr.tile([BP, N], F32, tag="lg")
            nc.sync.dma_start(lg[:], lgd[:].rearrange("p (b n) -> (p b) n", b=BP))
            wsg = scr.tile([BP, N], F32, tag="wsg")
            nc.scalar.activation(wsg[:], lg[:], AF.Sigmoid, bias=b4[:], scale=1.0)
            w = scr.tile([BP, N], F32, tag="w")
            nc.vector.tensor_scalar_add(w[:], wsg[:], 0.01)
            nc.sync.dma_start(wgt_out[:], w[:])

            px = scr.tile([BP, N], F32, tag="px")
            py = scr.tile([BP, N], F32, tag="py")
            pz_ = scr.tile([BP, N], F32, tag="pz")
            nc.sync.dma_start(px[:], pts_in[:, 0, :])
            nc.sync.dma_start(py[:], pts_in[:, 1, :])
            nc.sync.dma_start(pz_[:], pts_in[:, 2, :])

            wx = scr.tile([BP, N], F32, tag="wx")
            wy = scr.tile([BP, N], F32, tag="wy")
            wz = scr.tile([BP, N], F32, tag="wz")
            nc.vector.tensor_mul(wx[:], w[:], px[:])
            nc.vector.tensor_mul(wy[:], w[:], py[:])
            nc.vector.tensor_mul(wz[:], w[:], pz_[:])

            def fused_dot(in0, in1, name):
                # accum_out = sum(in0 * in1) in a single DVE op
                o = scr.tile([BP, N], F32, tag="ttr_scr", bufs=4)
                acc = tiny.tile([BP, 1], F32, tag=name)
                nc.vector.scalar_tensor_tensor(
                    o[:], in0=in0[:], scalar=1.0, in1=in1[:],
                    op0=ALU.mult, op1=ALU.mult, accum_out=acc[:])
                return acc

            def tred(in0, name):
                acc = tiny.tile([BP, 1], F32, tag=name)
                nc.vector.reduce_sum(acc[:], in0[:], axis=AX.X)
                return acc

            sxx = fused_dot(wx, px, "sxx")
            sxy = fused_dot(wx, py, "sxy")
            syy = fused_dot(wy, py, "syy")
            sxz = fused_dot(wx, pz_, "sxz")
            syz = fused_dot(wy, pz_, "syz")
            sx = tred(wx, "sx")
            sy = tred(wy, "sy")
            sz = tred(wz, "sz")
            sw = tred(w, "sw")

            def tt(op, a, b_, name):
                o = tiny.tile([BP, 1], F32, tag=name)
                nc.vector.tensor_tensor(o[:], a[:], b_[:], op)
                return o

            def fms(a, bs, t, name):
                # a * bs - t   (bs used as per-partition scalar operand)
                o = tiny.tile([BP, 1], F32, tag=name)
                nc.vector.scalar_tensor_tensor(
                    o[:], in0=a[:], scalar=bs[:], in1=t[:],
                    op0=ALU.mult, op1=ALU.subtract)
                return o

            def fma(a, bs, t, name):
                # a * bs + t
                o = tiny.tile([BP, 1], F32, tag=name)
                nc.vector.scalar_tensor_tensor(
                    o[:], in0=a[:], scalar=bs[:], in1=t[:],
                    op0=ALU.mult, op1=ALU.add)
                return o

            t_yy = tt(ALU.mult, sy, sy, "t_yy")
            c00 = fms(syy, sw, t_yy, "c00")
            t_xyw = tt(ALU.mult, sxy, sw, "t_xyw")
            c01 = fms(sy, sx, t_xyw, "c01")
            t_yyx = tt(ALU.mult, syy, sx, "t_yyx")
            c02 = fms(sxy, sy, t_yyx, "c02")
            t_xx = tt(ALU.mult, sx, sx, "t_xx")
            c11 = fms(sxx, sw, t_xx, "c11")
            t_xxy = tt(ALU.mult, sxx, sy, "t_xxy")
            c12 = fms(sx, sxy, t_xxy, "c12")
            t_xy2 = tt(ALU.mult, sxy, sxy, "t_xy2")
            c22 = fms(sxx, syy, t_xy2, "c22")

            def dot3(a0, b0, a1, b1, a2, b2, name):
                t1 = tt(ALU.mult, a1, b1, name + "_t1")
                t01 = fma(a0, b0, t1, name + "_t01")
                return fma(a2, b2, t01, name)

            det = dot3(sxx, c00, sxy, c01, sx, c02, "det")
            rdet = tiny.tile([BP, 1], F32, tag="rdet")
            nc.vector.reciprocal(rdet[:], det[:])

            nb0 = dot3(c00, sxz, c01, syz, c02, sz, "nb0")
            nb1 = dot3(c01, sxz, c11, syz, c12, sz, "nb1")
            nb2 = dot3(c02, sxz, c12, syz, c22, sz, "nb2")
            bt0 = tt(ALU.mult, nb0, rdet, "bt0")
            bt1 = tt(ALU.mult, nb1, rdet, "bt1")
            bt2 = tt(ALU.mult, nb2, rdet, "bt2")

            beta_t = tiny.tile([BP, 3], F32, tag="beta_t")
            nc.vector.tensor_copy(beta_t[:, 0:1], bt0[:])
            nc.vector.tensor_copy(beta_t[:, 1:2], bt1[:])
            nc.vector.tensor_copy(beta_t[:, 2:3], bt2[:])
            nc.sync.dma_start(beta_out[:], beta_t[:])

            # normal = [-b0, -b1, 1] / sqrt(b0^2 + b1^2 + 1)
            q1 = tt(ALU.mult, bt1, bt1, "q1")
            qs = fma(bt0, bt0, q1, "qs")
            nrm2 = tiny.tile([BP, 1], F32, tag="nrm2")
            nc.vector.tensor_scalar_add(nrm2[:], qs[:], 1.0)
            nrm = tiny.tile([BP, 1], F32, tag="nrm")
            nc.scalar.sqrt(nrm[:], nrm2[:])
            rinv = tiny.tile([BP, 1], F32, tag="rinv")
            nc.vector.reciprocal(rinv[:], nrm[:])
            neg_rinv = tiny.tile([BP, 1], F32, tag="neg_rinv")
            nc.vector.tensor_scalar_mul(neg_rinv[:], rinv[:], -1.0)

            nrm_t = tiny.tile([BP, 3], F32, tag="nrm_t")
            nc.vector.tensor_mul(nrm_t[:, 0:1], bt0[:], neg_rinv[:])
            nc.vector.tensor_mul(nrm_t[:, 1:2], bt1[:], neg_rinv[:])
            nc.vector.tensor_copy(nrm_t[:, 2:3], rinv[:])
            nc.sync.dma_start(nrm_out[:], nrm_t[:])


def _dummy_outputs(nc, tiny, nrm_out, beta_out, wgt_out):
    d3 = tiny.tile([BP, 3], mybir.dt.float32, tag="dump3")
    nc.vector.memset(d3[:], 0.0)
    nc.sync.dma_start(nrm_out[:], d3[:])
    nc.sync.dma_start(beta_out[:], d3[:])
    dn = tiny.tile([BP, N], mybir.dt.float32, tag="dumpn")
    nc.vector.memset(dn[:], 0.0)
    nc.sync.dma_start(wgt_out[:], dn[:])


def _bn_params(nc, tc, tiny, dram, st_list, g_t, be_t, eps_t, name):
    """Aggregate per-m-chunk bn_stats, AllReduce [mean, E[z^2]] across cores,
    produce scale [128, M] / bias [128, M] for the BN affine. Per-channel
    math is vectorized across the M chunks to keep the dependency chain
    short."""
    M = len(st_list)
    agg = tiny.tile([128, 2 * M], F32, tag=f"{name}_agg")
    for m in range(M):
        nc.vector.bn_aggr(agg[:, 2 * m:2 * m + 2], st_list[m][:])
    means = agg[:, 0:2 * M:2]
    varis = agg[:, 1:2 * M:2]
    # payload: [mean/8 ..., (var + mean^2)/8 ...] in [128, 2M]
    pay = tiny.tile([128, 2 * M], F32, tag=f"{name}_pay")
    nc.vector.tensor_scalar_mul(pay[:, 0:M], means, 1.0 / NCORES)
    ez2 = tiny.tile([128, M], F32, tag=f"{name}_ez2")
    nc.vector.tensor_tensor(ez2[:], means, means, ALU.mult)
    nc.vector.tensor_add(ez2[:], ez2[:], varis)
    nc.vector.tensor_scalar_mul(pay[:, M:2 * M], ez2[:], 1.0 / NCORES)

    cin = dram.tile([128, 2 * M], F32, name=f"{name}_cin")
    cout = dram.tile([128 * NCORES, 2 * M], F32, name=f"{name}_cout")
    nc.scalar.dma_start(cin[:], pay[:])
    nc.gpsimd.collective_compute(
        "AllGather", ALU.bypass,
        replica_groups=[list(range(NCORES))],
        ins=[cin.opt()], outs=[cout.opt()])
    arg = tiny.tile([128, NCORES * 2 * M], F32, tag=f"{name}_arg")
    nc.scalar.dma_start(
        arg[:].rearrange("p (r c) -> p r c", r=NCORES),
        cout[:].rearrange("(r p) c -> p r c", p=128))
    ar = tiny.tile([128, 2 * M], F32, tag=f"{name}_ar")
    nc.vector.reduce_sum(
        ar[:], arg[:].rearrange("p (r c) -> p c r", r=NCORES), axis=AX.X)

    gmean = ar[:, 0:M]
    gez2 = ar[:, M:2 * M]
    # var = E[z^2] - mean^2 ; rstd = 1/sqrt(var+eps)
    var = tiny.tile([128, M], F32, tag=f"{name}_var")
    nc.vector.tensor_tensor(var[:], gmean, gmean, ALU.mult)
    nc.vector.tensor_sub(var[:], gez2, var[:])
    std = tiny.tile([128, M], F32, tag=f"{name}_std")
    nc.scalar.activation(std[:], var[:], AF.Sqrt, bias=eps_t[:], scale=1.0)
    rstd = tiny.tile([128, M], F32, tag=f"{name}_rstd")
    nc.vector.reciprocal(rstd[:], std[:])
    sc = tiny.tile([128, M], F32, tag=f"{name}_sc")
    nc.vector.tensor_mul(sc[:], g_t[:, 0:M], rstd[:])
    bi = tiny.tile([128, M], F32, tag=f"{name}_bi")
    nc.vector.tensor_mul(bi[:], sc[:], gmean)
    nc.vector.tensor_sub(bi[:], be_t[:, 0:M], bi[:])
    return sc, bi


_NC_CACHE = {}


def _get_nc():
    if "nc" not in _NC_CACHE:
        _NC_CACHE["nc"] = _build()
    return _NC_CACHE["nc"]


def _prepare_in_maps(points, x, w1, w2, w3, w4, g1, be1, g2, be2, g3, be3, b4):
    points = np.asarray(points, dtype=np.float32)
    x = np.asarray(x, dtype=np.float32)
    w1 = np.asarray(w1, dtype=np.float32)
    w2 = np.asarray(w2, dtype=np.float32)
    w3 = np.asarray(w3, dtype=np.float32)
    w4 = np.asarray(w4, dtype=np.float32)
    g1 = np.asarray(g1, dtype=np.float32)
    be1 = np.asarray(be1, dtype=np.float32)
    g2 = np.asarray(g2, dtype=np.float32)
    be2 = np.asarray(be2, dtype=np.float32)
    g3 = np.asarray(g3, dtype=np.float32)
    be3 = np.asarray(be3, dtype=np.float32)
    b4 = np.asarray(b4, dtype=np.float32)
    # NOTE: b1/b2/b3 cancel exactly inside training-mode BatchNorm
    # (y - mean(y) is invariant to a per-channel constant shift), so they
    # are not sent to the device. n_effective_points is unused by the
    # reference module.

    shared = {
        "w1t": np.ascontiguousarray(w1.T).astype(np.float16),
        "w2t": np.ascontiguousarray(w2.T).astype(np.float16),
        "w3t": np.ascontiguousarray(w3.T).astype(np.float16),
        "w4c": np.ascontiguousarray(w4.reshape(128, 1)).astype(np.float16),
        "g1c": np.ascontiguousarray(g1.reshape(4, 128).T),
        "be1c": np.ascontiguousarray(be1.reshape(4, 128).T),
        "g2c": np.ascontiguousarray(g2.reshape(2, 128).T),
        "be2c": np.ascontiguousarray(be2.reshape(2, 128).T),
        "g3c": np.ascontiguousarray(g3.reshape(1, 128).T),
        "be3c": np.ascontiguousarray(be3.reshape(1, 128).T),
        "b4c": np.full((BP, 1), np.float32(b4.reshape(-1)[0]), dtype=np.float32),
    }
    in_maps = []
    for c in range(NCORES):
        sl = slice(c * BP, (c + 1) * BP)
        m = dict(shared)
        m["x"] = np.ascontiguousarray(x[sl]).astype(np.float16)
        m["points"] = np.ascontiguousarray(points[sl])
        in_maps.append(m)
    return in_maps


def _gather(res):
    normal = np.concatenate([res.results[c]["normal"] for c in range(NCORES)], axis=0)
    beta = np.concatenate([res.results[c]["beta"] for c in range(NCORES)], axis=0)
    weights = np.concatenate([res.results[c]["weights"] for c in range(NCORES)], axis=0)
    return normal.astype(np.float32), beta.astype(np.float32), weights.astype(np.float32)


def kernel(points, x, n_effective_points,
           w1, b1, g1, be1, w2, b2, g2, be2, w3, b3, g3, be3, w4, b4,
           **_unused):
    nc = _get_nc()
    in_maps = _prepare_in_maps(points, x, w1, w2, w3, w4,
                               g1, be1, g2, be2, g3, be3, b4)
    res = run_bass_kernel_spmd(nc, in_maps, core_ids=list(range(NCORES)))
    _NC_CACHE["last_results"] = res
    return _gather(res)
